# revision 41
# baseline (speedup 1.0000x reference)
"""Trainium2 Bass kernel for nn_Attention_12266426598027.

GQA attention layer (B=4, S=2048, H=896, 14 q-heads / 2 kv-heads, HD=64,
RoPE theta=1e6, causal) distributed over 8 NeuronCores.

Sharding: core = (batch b, kv-group g) with b in 0..3, g in 0..1. Each core
computes 7 q-heads against its kv head for one batch, including its slice of
the QKV projection and a partial o_proj (448 of the 896 contraction dims).
The two partial o_proj outputs per batch are summed on the host (the
"all-reduce after o_proj" of the tensor-parallel split).

Measured-HW design notes:
- The PE dual-issues matmuls whose stationary tiles sit on disjoint row
  halves (tile_position row 0 vs 64): K=64 scores matmuls run at ~111-136ns
  per 512 cols when emitted as even/odd head ping-pong pairs vs ~420ns
  alone. Head 6 ping-pongs on k-chunk parity against duplicated q6/k rows.
- Matmul slices run ~2x slower in the full kernel than in isolation due to
  intra-core SBUF bandwidth contention with ACT/DVE/DMA traffic, so the
  whole data path is bf16 (2 bytes/elem) except PSUM accumulations, the
  softmax normalization math, and the final f32 output. Measured rel err
  ~2e-3 vs the 2e-2 gate.
- Phase C is ACT(exp)-bound: scores land in [128,1536] PSUM tiles (3 banks,
  one exp instruction per 3 k-chunks per head) to amortize the ~235ns
  per-instruction ACT overhead. Causal masking is applied after exp by
  zeroing above-diagonal triangles of the probs on the Pool engine.
- Rowsums ride as a 65th ones-column on V; normalization is a DVE rowsum
  copy + reciprocal_approx_fast (custom DVE op; needs SBUF input) + Pool
  partition_broadcast + one DVE multiply.
- o_proj for q-block j-1 is interleaved at pair boundaries inside block j
  to fill PE bubbles; its PSUM tiles share the pv tag (2 banks total).
- Phase A: m-tile order [k;q6] first (gates attention), v last (needs no
  RoPE; transposed into v_all by bf16 PE transposes). RoPE rotate-half
  swaps are full-row sync-issued DMAs (one HOL-wait per m-tile; gpsimd
  dma_start would run as DIRECT2D *on* the Pool engine and serialize);
  combines on DVE in 2-byte mode, overlapped with A's matmul stream.
  Bias-adds ride on ACT. Head pair (6,None) runs third in phase C since
  q6's rope (m0) lands long before (4,5)'s (m3).
"""
import sys

for _p in ('/opt/trn_rl_repo', '/root/.axon_site'):
    if _p not in sys.path:
        sys.path.insert(0, _p)

import numpy as np

B, S, H = 4, 2048, 896
NH, NKV, HD = 14, 2, 64
NHC, DQ = 7, 448          # q-heads per core, their stacked dim
ROPE_THETA = 1e6

_PROGRAM_CACHE = {}


def _build_program():
    import concourse.bass as bass
    from concourse import bacc
    import concourse.mybir as mybir
    import concourse.tile as tile
    F32 = mybir.dt.float32
    F32R = mybir.dt.float32r
    BF16 = mybir.dt.bfloat16
    ALU = mybir.AluOpType
    AF = mybir.ActivationFunctionType

    nc = bacc.Bacc("TRN2", target_bir_lowering=False, debug=False, num_devices=8)

    xT_d = nc.dram_tensor("xT", [H, S], BF16, kind="ExternalInput").ap()
    # wT columns: [k(64) | q6(64) | q0..q5(384) | v(64)]  (576 total)
    wT_d = nc.dram_tensor("wT", [H, 576], BF16, kind="ExternalInput").ap()
    bias_d = nc.dram_tensor("bias", [640], F32, kind="ExternalInput").ap()
    woT_d = nc.dram_tensor("woT", [DQ, H], BF16, kind="ExternalInput").ap()
    cos2_d = nc.dram_tensor("cos2", [128, S], BF16, kind="ExternalInput").ap()
    sinm2_d = nc.dram_tensor("sinm2", [128, S], BF16, kind="ExternalInput").ap()
    ident_d = nc.dram_tensor("ident64", [64, 64], BF16, kind="ExternalInput").ap()
    yT_d = nc.dram_tensor("yT", [H, S], F32, kind="ExternalOutput").ap()
    import os as _os
    DEBUG = _os.environ.get("KERNEL_DEBUG_OUTPUTS", "0") == "1"
    if DEBUG:
        dbg = {}
        for nm, shp in [("dqkv", [5 * 128, S]), ("dqr", [4 * 128, S]),
                        ("dk2", [128, S]), ("dv", [128, 16 * 65]),
                        ("dattn", [4 * 128, S])]:
            dbg[nm] = nc.dram_tensor(nm, shp, F32, kind="ExternalOutput").ap()

    with tile.TileContext(nc) as tc:
        with tc.tile_pool(name="persist", bufs=1) as pp, \
             tc.tile_pool(name="small", bufs=1) as psm:

            # persistent SBUF tensors (all bf16)
            qr = [pp.tile([128, S], BF16, tag=f"qr{m}", name=f"qr{m}")
                  for m in range(4)]       # qr0..2: q-pairs; qr3: q6 (dup'd)
            k2 = pp.tile([128, S], BF16, tag="k2", name="k2")
            v_all = pp.tile([128, 16 * 65], BF16, tag="v_all", name="v_all")
            attn_all = [pp.tile([128, S], BF16, tag=f"attn{i}",
                                name=f"attn{i}") for i in range(4)]
            cos2t = pp.tile([128, S], BF16, tag="cos2t", name="cos2t")
            sinm2t = pp.tile([128, S], BF16, tag="sinm2t", name="sinm2t")

            biast = psm.tile([128, 5], F32, name="biast")
            ident = psm.tile([64, 64], BF16, name="ident")

            # ones columns for the rowsum trick (v data cols overwritten later)
            nc.vector.memset(v_all[:], 1.0)

            # ---- phase A: QKV projection + B: RoPE/v-transpose -----------
            with tc.tile_pool(name="ioA", bufs=1) as pio, \
                 tc.tile_pool(name="psA", bufs=1, space="PSUM") as psA:
                wt = [pio.tile([128, 576], BF16, tag=f"w{i}", name=f"w{i}")
                      for i in range(7)]
                xt = [pio.tile([128, S], BF16, tag=f"x{i}", name=f"x{i}")
                      for i in range(7)]
                # x tiles first (they gate the first matmul chain), on the
                # ACT issue queue; everything else on SP.
                for i in range(7):
                    nc.scalar.dma_start(xt[i][:], xT_d[128 * i:128 * i + 128, :])
                for i in range(7):
                    nc.sync.dma_start(wt[i][:], wT_d[128 * i:128 * i + 128, :])
                nc.sync.dma_start(biast[:], bias_d.rearrange("(m p) -> p m", p=128))
                nc.sync.dma_start(cos2t[:], cos2_d[:])
                nc.sync.dma_start(sinm2t[:], sinm2_d[:])
                nc.sync.dma_start(ident[:], ident_d[:])

                def rope_full(src, dst, rows, nm):
                    """RoPE src[rows, :] -> dst[rows, :] over the full row.
                    rotate-half swap via [32,S] DMA pieces on SP (one
                    HOL-wait per m-tile), combines on DVE (2-byte mode)."""
                    r0, r1 = rows
                    xsw = pio.tile([128, S], BF16, tag="xsw", bufs=2,
                                   name=f"xsw{nm}")
                    for base in range(r0, r1, 64):
                        nc.sync.dma_start(xsw[base:base + 32, :],
                                          src[base + 32:base + 64, :])
                        nc.sync.dma_start(xsw[base + 32:base + 64, :],
                                          src[base:base + 32, :])
                    tsin = pio.tile([128, S], BF16, tag="tsin", bufs=2,
                                    name=f"tsin{nm}")
                    nc.vector.tensor_tensor(tsin[r0:r1, :], xsw[r0:r1, :],
                                            sinm2t[r0:r1, :], ALU.mult)
                    nc.vector.tensor_tensor(dst[r0:r1, :], src[r0:r1, :],
                                            cos2t[r0:r1, :], ALU.mult)
                    nc.vector.tensor_tensor(dst[r0:r1, :], dst[r0:r1, :],
                                            tsin[r0:r1, :], ALU.add)

                # m-tiles: m0=[k;q6], m1=[q0;q1], m2=[q2;q3], m3=[q4;q5],
                # m4=[v;pad]
                qkv = []
                M_SIZES = [128, 128, 128, 128, 64]
                for m in range(5):
                    M, mo = M_SIZES[m], 128 * m
                    qm = pio.tile([128, S], BF16, tag="qkv", bufs=4,
                                  name=f"qkv{m}")
                    qkv.append(qm)
                    for sc in range(4):
                        ps = psA.tile([128, 512], F32, tag="qkvps", bufs=6,
                                      name=f"psA{m}_{sc}")
                        for h in range(7):
                            nc.tensor.matmul(
                                ps[0:M, :],
                                wt[h][:, mo:mo + M],
                                xt[h][:, 512 * sc:512 * sc + 512],
                                start=(h == 0), stop=(h == 6))
                        nc.scalar.activation(
                            qm[0:M, 512 * sc:512 * sc + 512], ps[0:M, :],
                            AF.Identity, bias=biast[0:M, m:m + 1], scale=1.0)
                        if m == 4:
                            # v: PE transpose (bf16) into v_all (no RoPE)
                            for i in range(4 * sc, 4 * sc + 4):
                                pst = psA.tile([128, 64], BF16, tag="vtr",
                                               bufs=2, name=f"vtr{i}")
                                nc.tensor.transpose(
                                    pst[:], qm[0:64, 128 * i:128 * i + 128],
                                    ident[:])
                                nc.vector.tensor_copy(
                                    v_all[:, 65 * i:65 * i + 64], pst[:])

                    if m == 0:
                        # k (rows 0:64) -> k2 low; q6 (rows 64:128) -> qr3
                        # high; one fused full-row rope, then row-half dups.
                        kq = pio.tile([128, S], BF16, tag="kq", name="kq")
                        rope_full(qm, kq, (0, 128), "k")
                        nc.sync.dma_start(k2[0:64, :], kq[0:64, :])
                        nc.sync.dma_start(k2[64:128, :], kq[0:64, :])
                        nc.sync.dma_start(qr[3][64:128, :], kq[64:128, :])
                        nc.sync.dma_start(qr[3][0:64, :], kq[64:128, :])
                    elif m <= 3:
                        rope_full(qm, qr[m - 1], (0, 128), f"q{m}")

                if DEBUG:
                    dstage = pp.tile([128, S], F32, tag="dstage",
                                     name="dstage")
                    for m in range(5):
                        nc.vector.tensor_copy(dstage[:], qkv[m][:])
                        nc.sync.dma_start(
                            dbg["dqkv"][128 * m:128 * m + 128, :], dstage[:])

            if DEBUG:
                dstage2 = pp.tile([128, S], F32, tag="dstage2", name="dstage2")
                for m in range(4):
                    nc.vector.tensor_copy(dstage2[:], qr[m][:])
                    nc.sync.dma_start(dbg["dqr"][128 * m:128 * m + 128, :],
                                      dstage2[:])
                nc.vector.tensor_copy(dstage2[:], k2[:])
                nc.sync.dma_start(dbg["dk2"][:], dstage2[:])
                nc.vector.tensor_copy(dstage2[:, 0:16 * 65], v_all[:])
                nc.sync.dma_start(dbg["dv"][:], dstage2[:, 0:16 * 65])

            # ---- phases C+D: attention + o_proj --------------------------
            with tc.tile_pool(name="psC", bufs=1, space="PSUM") as psC:
                wo = [pp.tile([128, H], BF16, tag=f"wo{i}", name=f"wo{i}")
                      for i in range(4)]
                for cc in range(4):
                    K = 128 if cc < 3 else 64
                    nc.sync.dma_start(wo[cc][0:K, :],
                                      woT_d[128 * cc:128 * cc + K, :])

                def emit_oproj(j, ots):
                    """o_proj for q-block j, output tiles `ots`."""
                    qs = slice(512 * j, 512 * j + 512)
                    for ot in ots:
                        py = psC.tile([128, 512], F32, tag="pvy", bufs=2,
                                      name=f"py{j}_{ot}")
                        for cc in range(4):
                            K = 128 if cc < 3 else 64
                            nc.tensor.matmul(
                                py[:],
                                wo[cc][0:K, 128 * ot:128 * ot + 128],
                                attn_all[cc][0:K, qs],
                                start=(cc == 0), stop=(cc == 3))
                        ysb = pp.tile([128, 512], F32, tag="ysb", bufs=3,
                                        name=f"ysb{j}_{ot}")
                        nc.vector.tensor_copy(ysb[:], py[:])
                        nc.sync.dma_start(
                            yT_d[128 * ot:128 * ot + 128, qs], ysb[:])

                # head -> (q tile, row half) ; scores ping-pong on row halves
                def score_ops(h, c):
                    if h < 6:
                        row = 64 * (h % 2)
                        qt = qr[h // 2]
                    else:
                        row = 64 * (c % 2)      # chunk-parity ping-pong
                        qt = qr[3]
                    return qt, row

                # (6,None) third: q6 is roped with m0, so it is ready
                # before (4,5), whose rope (m3) lands at the tail of phase A
                PAIRS = [(0, 1), (2, 3), (6, None), (4, 5)]

                for j in range(4):
                    nkc = 4 * j + 4
                    qs = slice(512 * j, 512 * j + 512)
                    groups = [list(range(s, min(s + 3, nkc)))
                              for s in range(0, nkc, 3)]
                    for ip, pair in enumerate(PAIRS):
                        heads = [h for h in pair if h is not None]
                        pv = {h: psC.tile([65, 512], F32, tag="pvy", bufs=2,
                                          name=f"pv{j}_{h}")
                              for h in heads}

                        def emit_pv(grp, probs_of):
                            for h in heads:
                                pr = probs_of[h]
                                for i, c in enumerate(grp):
                                    t = c - 4 * j
                                    lo = 0 if t < 1 else min(128 * t, 256)
                                    nc.tensor.matmul(
                                        pv[h][:, lo:512],
                                        v_all[:, 65 * c:65 * c + 65],
                                        pr[:, 512 * i + lo:512 * i + 512],
                                        start=(c == 0), stop=(c == nkc - 1))

                        prev = None
                        for grp in groups:
                            ncols = 512 * len(grp)
                            sct = {h: psC.tile([128, 1536], F32, tag="sc",
                                               bufs=2,
                                               name=f"sc{j}_{h}_{grp[0]}")
                                   for h in heads}
                            # scores: even/odd row-half ping-pong per chunk
                            for c in grp:
                                for h in heads:
                                    qt, row = score_ops(h, c)
                                    i = c - grp[0]
                                    nc.tensor.matmul(
                                        sct[h][:, 512 * i:512 * i + 512],
                                        k2[row:row + 64, 128 * c:128 * c + 128],
                                        qt[row:row + 64, qs],
                                        start=True, stop=True)
                            probs_of = {}
                            for h in heads:
                                probs = pp.tile([128, 1536], BF16,
                                                  tag="probs", bufs=8,
                                                  name=f"pr{j}_{h}_{grp[0]}")
                                probs_of[h] = probs
                                nc.scalar.activation(
                                    probs[:, 0:ncols], sct[h][:, 0:ncols],
                                    AF.Exp, bias=0.0, scale=0.125)
                                # zero above-diagonal triangles (diag chunks)
                                for i, c in enumerate(grp):
                                    t = c - 4 * j
                                    if t < 0:
                                        continue
                                    if t == 3:
                                        nc.gpsimd.memset(
                                            probs[:, 512 * i + 256:
                                                  512 * i + 384], 0.0)
                                    nc.gpsimd.affine_select(
                                        out=probs[:, 512 * i + 128 * t:
                                                  512 * i + 128 * t + 128],
                                        in_=probs[:, 512 * i + 128 * t:
                                                  512 * i + 128 * t + 128],
                                        compare_op=ALU.is_ge, fill=0.0,
                                        base=0, pattern=[[1, 128]],
                                        channel_multiplier=-1)
                            if prev is not None:
                                emit_pv(*prev)
                            prev = (grp, probs_of)
                        emit_pv(*prev)
                        # normalize: attn = pv[0:64] / rowsum (pv row 64)
                        for h in heads:
                            rsum = pp.tile([1, 512], F32, tag="rsum",
                                             bufs=4, name=f"rs{j}_{h}")
                            nc.vector.tensor_copy(rsum[:], pv[h][64:65, :])
                            rcp = pp.tile([1, 512], F32, tag="rcp", bufs=4,
                                            name=f"rcp{j}_{h}")
                            nc.vector.reciprocal_approx_fast(
                                out=rcp[:], in_=rsum[:])
                            rb = pp.tile([64, 512], F32, tag="rb", bufs=4,
                                           name=f"rb{j}_{h}")
                            nc.gpsimd.partition_broadcast(rb[:], rcp[:])
                            dst = attn_all[h // 2][
                                64 * (h % 2):64 * (h % 2) + 64, qs]
                            nc.vector.tensor_tensor(dst, pv[h][0:64, :],
                                                    rb[:], ALU.mult)
                        # interleave previous block's o_proj into PE bubbles
                        if j >= 1:
                            emit_oproj(j - 1,
                                       [[0, 1], [2, 3], [4, 5], [6]][ip])
                if DEBUG:
                    dstage3 = pp.tile([128, S], F32, tag="dstage3",
                                        name="dstage3")
                    for i in range(4):
                        nc.vector.tensor_copy(dstage3[:], attn_all[i][:])
                        nc.sync.dma_start(
                            dbg["dattn"][128 * i:128 * i + 128, :],
                            dstage3[:])
                emit_oproj(3, list(range(7)))

    nc.compile()
    return nc


def _host_prep(inputs):
    import ml_dtypes
    BF = ml_dtypes.bfloat16
    hid = np.ascontiguousarray(np.asarray(inputs["hidden_states"], np.float32))
    pos = np.asarray(inputs["position_ids"])[0].astype(np.float32)
    Wq = np.asarray(inputs["Wq"], np.float32)
    bq = np.asarray(inputs["bq"], np.float32)
    Wk = np.asarray(inputs["Wk"], np.float32)
    bk = np.asarray(inputs["bk"], np.float32)
    Wv = np.asarray(inputs["Wv"], np.float32)
    bv = np.asarray(inputs["bv"], np.float32)
    Wo = np.asarray(inputs["Wo"], np.float32)

    inv = (1.0 / (ROPE_THETA ** (np.arange(0, HD, 2, dtype=np.float32) / HD))
           ).astype(np.float32)
    freqs = pos[:, None] * inv[None, :]
    emb = np.concatenate([freqs, freqs], -1)            # [S, 64]
    cosT = np.cos(emb).T.astype(np.float32)             # [64, S]
    sinT = np.sin(emb).T.astype(np.float32)
    sinm = sinT.copy()
    sinm[0:32] *= -1.0                                  # fold rotate_half sign
    cos2 = np.ascontiguousarray(np.vstack([cosT, cosT])).astype(BF)
    sinm2 = np.ascontiguousarray(np.vstack([sinm, sinm])).astype(BF)

    maps = []
    for b in range(B):
        for g in range(2):
            xT = np.ascontiguousarray(hid[b].T).astype(BF)
            # column blocks: [k(64) | q6(64) | q0..q5(384) | v(64)]
            Wsl = np.concatenate([Wk[64 * g:64 * g + 64],
                                  Wq[448 * g + 384:448 * g + 448],
                                  Wq[448 * g:448 * g + 384],
                                  Wv[64 * g:64 * g + 64]], 0)
            wT = np.ascontiguousarray(Wsl.T).astype(BF)  # [896, 576]
            bias = np.zeros(640, np.float32)
            bias[:576] = np.concatenate([bk[64 * g:64 * g + 64],
                                         bq[448 * g + 384:448 * g + 448],
                                         bq[448 * g:448 * g + 384],
                                         bv[64 * g:64 * g + 64]])
            woT = np.ascontiguousarray(
                Wo[:, 448 * g:448 * g + 448].T).astype(BF)
            maps.append(dict(xT=xT, wT=wT, bias=bias, woT=woT,
                             cos2=cos2, sinm2=sinm2,
                             ident64=np.eye(64, dtype=BF)))
    return maps


def kernel(**inputs) -> np.ndarray:
    from concourse.bass_utils import run_bass_kernel_spmd

    if "nc" not in _PROGRAM_CACHE:
        _PROGRAM_CACHE["nc"] = _build_program()
    nc = _PROGRAM_CACHE["nc"]

    in_maps = _host_prep(inputs)
    res = run_bass_kernel_spmd(nc, in_maps, core_ids=list(range(8)),
                               **_PROGRAM_CACHE.get("run_kwargs", {}))
    _PROGRAM_CACHE["last_result"] = res
    yTs = [res.results[i]["yT"] for i in range(8)]
    out = np.stack([(yTs[2 * b] + yTs[2 * b + 1]).T for b in range(B)], 0)
    return np.ascontiguousarray(out)


# revision 42
# speedup vs baseline: 1.1981x; 1.1981x over previous
"""Trainium2 Bass kernel for nn_Attention_12266426598027.

GQA attention layer (B=4, S=2048, H=896, 14 q-heads / 2 kv-heads, HD=64,
RoPE theta=1e6, causal) distributed over 8 NeuronCores.

Sharding: core = (batch b, kv-group g) with b in 0..3, g in 0..1. Each core
computes 7 q-heads against its kv head for one batch, including its slice of
the QKV projection and a partial o_proj (448 of the 896 contraction dims).
The two partial o_proj outputs per batch are summed on the host (the
"all-reduce after o_proj" of the tensor-parallel split).

Measured-HW design notes:
- The PE dual-issues matmuls whose stationary tiles sit on disjoint row
  halves (tile_position row 0 vs 64): K=64 scores matmuls run at ~111-136ns
  per 512 cols when emitted as even/odd head ping-pong pairs vs ~420ns
  alone. Head 6 ping-pongs on k-chunk parity against duplicated q6/k rows.
- Matmul slices run ~2x slower in the full kernel than in isolation due to
  intra-core SBUF bandwidth contention with ACT/DVE/DMA traffic, so the
  whole data path is bf16 (2 bytes/elem) except PSUM accumulations, the
  softmax normalization math, and the final f32 output. Measured rel err
  ~2e-3 vs the 2e-2 gate.
- Phase C is ACT(exp)-bound: scores land in [128,1536] PSUM tiles (3 banks,
  one exp instruction per 3 k-chunks per head) to amortize the ~235ns
  per-instruction ACT overhead. Causal masking is applied after exp by
  zeroing above-diagonal triangles of the probs on the Pool engine.
- Rowsums ride as a 65th ones-column on V; normalization is a DVE rowsum
  copy + reciprocal_approx_fast (custom DVE op; needs SBUF input) + Pool
  partition_broadcast + one DVE multiply.
- o_proj for q-block j-1 is interleaved at pair boundaries inside block j
  to fill PE bubbles; its PSUM tiles share the pv tag (2 banks total).
- Phase A: m-tile order [k;q6] first (gates attention), v last (needs no
  RoPE; transposed into v_all by bf16 PE transposes). RoPE rotate-half
  swaps are full-row sync-issued DMAs (one HOL-wait per m-tile; gpsimd
  dma_start would run as DIRECT2D *on* the Pool engine and serialize);
  combines on DVE in 2-byte mode, overlapped with A's matmul stream.
  Bias-adds ride on ACT. Head pair (6,None) runs third in phase C since
  q6's rope (m0) lands long before (4,5)'s (m3).
"""
import sys

for _p in ('/opt/trn_rl_repo', '/root/.axon_site'):
    if _p not in sys.path:
        sys.path.insert(0, _p)

import numpy as np

B, S, H = 4, 2048, 896
NH, NKV, HD = 14, 2, 64
NHC, DQ = 7, 448          # q-heads per core, their stacked dim
ROPE_THETA = 1e6

_PROGRAM_CACHE = {}


def _build_program():
    import concourse.bass as bass
    from concourse import bacc
    import concourse.mybir as mybir
    import concourse.tile as tile
    F32 = mybir.dt.float32
    F32R = mybir.dt.float32r
    BF16 = mybir.dt.bfloat16
    ALU = mybir.AluOpType
    AF = mybir.ActivationFunctionType

    nc = bacc.Bacc("TRN2", target_bir_lowering=False, debug=False, num_devices=8)

    xT_d = nc.dram_tensor("xT", [H, S], BF16, kind="ExternalInput").ap()
    # wT columns: [k(64) | q6(64) | q0..q5(384) | v(64)]  (576 total)
    wT_d = nc.dram_tensor("wT", [H, 576], BF16, kind="ExternalInput").ap()
    bias_d = nc.dram_tensor("bias", [640], F32, kind="ExternalInput").ap()
    woT_d = nc.dram_tensor("woT", [DQ, H], BF16, kind="ExternalInput").ap()
    cos2_d = nc.dram_tensor("cos2", [128, S], BF16, kind="ExternalInput").ap()
    sinm2_d = nc.dram_tensor("sinm2", [128, S], BF16, kind="ExternalInput").ap()
    ident_d = nc.dram_tensor("ident64", [64, 64], BF16, kind="ExternalInput").ap()
    yT_d = nc.dram_tensor("yT", [H, S], F32, kind="ExternalOutput").ap()
    import os as _os
    DEBUG = _os.environ.get("KERNEL_DEBUG_OUTPUTS", "0") == "1"
    if DEBUG:
        dbg = {}
        for nm, shp in [("dqkv", [5 * 128, S]), ("dqr", [4 * 128, S]),
                        ("dk2", [128, S]), ("dv", [128, 16 * 65]),
                        ("dattn", [4 * 128, S])]:
            dbg[nm] = nc.dram_tensor(nm, shp, F32, kind="ExternalOutput").ap()

    with tile.TileContext(nc) as tc:
        with tc.tile_pool(name="persist", bufs=1) as pp, \
             tc.tile_pool(name="small", bufs=1) as psm:

            # persistent SBUF tensors (all bf16)
            qr = [pp.tile([128, S], BF16, tag=f"qr{m}", name=f"qr{m}")
                  for m in range(4)]       # qr0..2: q-pairs; qr3: q6 (dup'd)
            k2 = pp.tile([128, S], BF16, tag="k2", name="k2")
            v_all = pp.tile([128, 16 * 65], BF16, tag="v_all", name="v_all")
            attn_all = [pp.tile([128, S], BF16, tag=f"attn{i}",
                                name=f"attn{i}") for i in range(4)]
            cos2t = pp.tile([128, S], BF16, tag="cos2t", name="cos2t")
            sinm2t = pp.tile([128, S], BF16, tag="sinm2t", name="sinm2t")

            biast = psm.tile([128, 5], F32, name="biast")
            ident = psm.tile([64, 64], BF16, name="ident")

            # ones columns for the rowsum trick (v data cols overwritten later)
            nc.vector.memset(v_all[:], 1.0)

            # ---- phase A: QKV projection + B: RoPE/v-transpose -----------
            with tc.tile_pool(name="ioA", bufs=1) as pio, \
                 tc.tile_pool(name="psA", bufs=1, space="PSUM") as psA:
                wt = [pio.tile([128, 576], BF16, tag=f"w{i}", name=f"w{i}")
                      for i in range(7)]
                xt = [pio.tile([128, S], BF16, tag=f"x{i}", name=f"x{i}")
                      for i in range(7)]
                # x tiles first (they gate the first matmul chain), on the
                # ACT issue queue; everything else on SP.
                for i in range(7):
                    nc.scalar.dma_start(xt[i][:], xT_d[128 * i:128 * i + 128, :])
                for i in range(7):
                    nc.sync.dma_start(wt[i][:], wT_d[128 * i:128 * i + 128, :])
                nc.sync.dma_start(biast[:], bias_d.rearrange("(m p) -> p m", p=128))
                nc.sync.dma_start(cos2t[:], cos2_d[:])
                nc.sync.dma_start(sinm2t[:], sinm2_d[:])
                nc.sync.dma_start(ident[:], ident_d[:])

                def rope_full(src, dst, rows, nm):
                    """RoPE src[rows, :] -> dst[rows, :] over the full row.
                    rotate-half swap via [32,S] DMA pieces on SP (one
                    HOL-wait per m-tile), combines on DVE (2-byte mode)."""
                    r0, r1 = rows
                    xsw = pio.tile([128, S], BF16, tag="xsw", bufs=2,
                                   name=f"xsw{nm}")
                    for base in range(r0, r1, 64):
                        nc.sync.dma_start(xsw[base:base + 32, :],
                                          src[base + 32:base + 64, :])
                        nc.sync.dma_start(xsw[base + 32:base + 64, :],
                                          src[base:base + 32, :])
                    tsin = pio.tile([128, S], BF16, tag="tsin", bufs=2,
                                    name=f"tsin{nm}")
                    nc.vector.tensor_tensor(tsin[r0:r1, :], xsw[r0:r1, :],
                                            sinm2t[r0:r1, :], ALU.mult)
                    nc.vector.tensor_tensor(dst[r0:r1, :], src[r0:r1, :],
                                            cos2t[r0:r1, :], ALU.mult)
                    nc.vector.tensor_tensor(dst[r0:r1, :], dst[r0:r1, :],
                                            tsin[r0:r1, :], ALU.add)

                # m-tiles: m0=[k;q6], m1=[q0;q1], m2=[q2;q3], m3=[q4;q5],
                # m4=[v;pad]
                qkv = []
                M_SIZES = [128, 128, 128, 128, 64]
                for m in range(5):
                    M, mo = M_SIZES[m], 128 * m
                    qm = pio.tile([128, S], BF16, tag="qkv", bufs=4,
                                  name=f"qkv{m}")
                    qkv.append(qm)
                    for sc in range(4):
                        ps = psA.tile([128, 512], F32, tag="qkvps", bufs=6,
                                      name=f"psA{m}_{sc}")
                        for h in range(7):
                            nc.tensor.matmul(
                                ps[0:M, :],
                                wt[h][:, mo:mo + M],
                                xt[h][:, 512 * sc:512 * sc + 512],
                                start=(h == 0), stop=(h == 6))
                        nc.scalar.activation(
                            qm[0:M, 512 * sc:512 * sc + 512], ps[0:M, :],
                            AF.Identity, bias=biast[0:M, m:m + 1], scale=1.0)
                        if m == 4:
                            # v: PE transpose (bf16) into v_all (no RoPE)
                            for i in range(4 * sc, 4 * sc + 4):
                                pst = psA.tile([128, 64], BF16, tag="vtr",
                                               bufs=2, name=f"vtr{i}")
                                nc.tensor.transpose(
                                    pst[:], qm[0:64, 128 * i:128 * i + 128],
                                    ident[:])
                                nc.vector.tensor_copy(
                                    v_all[:, 65 * i:65 * i + 64], pst[:])

                    if m == 0:
                        # k (rows 0:64) -> k2 low; q6 (rows 64:128) -> qr3
                        # high; one fused full-row rope, then row-half dups.
                        kq = pio.tile([128, S], BF16, tag="kq", name="kq")
                        rope_full(qm, kq, (0, 128), "k")
                        nc.sync.dma_start(k2[0:64, :], kq[0:64, :])
                        nc.sync.dma_start(k2[64:128, :], kq[0:64, :])
                        nc.sync.dma_start(qr[3][64:128, :], kq[64:128, :])
                        nc.sync.dma_start(qr[3][0:64, :], kq[64:128, :])
                    elif m <= 3:
                        rope_full(qm, qr[m - 1], (0, 128), f"q{m}")

                if DEBUG:
                    dstage = pp.tile([128, S], F32, tag="dstage",
                                     name="dstage")
                    for m in range(5):
                        nc.vector.tensor_copy(dstage[:], qkv[m][:])
                        nc.sync.dma_start(
                            dbg["dqkv"][128 * m:128 * m + 128, :], dstage[:])

            if DEBUG:
                dstage2 = pp.tile([128, S], F32, tag="dstage2", name="dstage2")
                for m in range(4):
                    nc.vector.tensor_copy(dstage2[:], qr[m][:])
                    nc.sync.dma_start(dbg["dqr"][128 * m:128 * m + 128, :],
                                      dstage2[:])
                nc.vector.tensor_copy(dstage2[:], k2[:])
                nc.sync.dma_start(dbg["dk2"][:], dstage2[:])
                nc.vector.tensor_copy(dstage2[:, 0:16 * 65], v_all[:])
                nc.sync.dma_start(dbg["dv"][:], dstage2[:, 0:16 * 65])

            # ---- phases C+D: attention + o_proj --------------------------
            with tc.tile_pool(name="psC", bufs=1, space="PSUM") as psC:
                wo = [pp.tile([128, H], BF16, tag=f"wo{i}", name=f"wo{i}")
                      for i in range(4)]
                for cc in range(4):
                    K = 128 if cc < 3 else 64
                    nc.sync.dma_start(wo[cc][0:K, :],
                                      woT_d[128 * cc:128 * cc + K, :])

                def emit_oproj(j, ots):
                    """o_proj for q-block j, output tiles `ots`."""
                    qs = slice(512 * j, 512 * j + 512)
                    for ot in ots:
                        py = psC.tile([128, 512], F32, tag="pvy", bufs=2,
                                      name=f"py{j}_{ot}")
                        for cc in range(4):
                            K = 128 if cc < 3 else 64
                            nc.tensor.matmul(
                                py[:],
                                wo[cc][0:K, 128 * ot:128 * ot + 128],
                                attn_all[cc][0:K, qs],
                                start=(cc == 0), stop=(cc == 3))
                        ysb = pp.tile([128, 512], F32, tag="ysb", bufs=2,
                                        name=f"ysb{j}_{ot}")
                        nc.vector.tensor_copy(ysb[:], py[:])
                        nc.sync.dma_start(
                            yT_d[128 * ot:128 * ot + 128, qs], ysb[:])

                # head -> (q tile, row half) ; scores ping-pong on row halves
                def score_ops(h, c):
                    if h < 6:
                        row = 64 * (h % 2)
                        qt = qr[h // 2]
                    else:
                        row = 64 * (c % 2)      # chunk-parity ping-pong
                        qt = qr[3]
                    return qt, row

                # (6,None) third: q6 is roped with m0, so it is ready
                # before (4,5), whose rope (m3) lands at the tail of phase A
                PAIRS = [(0, 1), (2, 3), (6, None), (4, 5)]

                for j in range(4):
                    nkc = 4 * j + 4
                    qs = slice(512 * j, 512 * j + 512)
                    groups = [list(range(s, min(s + 3, nkc)))
                              for s in range(0, nkc, 3)]
                    for ip, pair in enumerate(PAIRS):
                        heads = [h for h in pair if h is not None]
                        pv = {h: psC.tile([65, 512], F32, tag="pvy", bufs=2,
                                          name=f"pv{j}_{h}")
                              for h in heads}

                        def emit_pv(grp, probs_of):
                            for h in heads:
                                pr = probs_of[h]
                                for i, c in enumerate(grp):
                                    t = c - 4 * j
                                    lo = 0 if t < 1 else min(128 * t, 256)
                                    nc.tensor.matmul(
                                        pv[h][:, lo:512],
                                        v_all[:, 65 * c:65 * c + 65],
                                        pr[:, 512 * i + lo:512 * i + 512],
                                        start=(c == 0), stop=(c == nkc - 1))

                        prev = None
                        for grp in groups:
                            ncols = 512 * len(grp)
                            sct = {h: psC.tile([128, 1536], F32, tag="sc",
                                               bufs=2,
                                               name=f"sc{j}_{h}_{grp[0]}")
                                   for h in heads}
                            # scores: even/odd row-half ping-pong per chunk
                            for c in grp:
                                for h in heads:
                                    qt, row = score_ops(h, c)
                                    i = c - grp[0]
                                    nc.tensor.matmul(
                                        sct[h][:, 512 * i:512 * i + 512],
                                        k2[row:row + 64, 128 * c:128 * c + 128],
                                        qt[row:row + 64, qs],
                                        start=True, stop=True)
                            probs_of = {}
                            for h in heads:
                                probs = pp.tile([128, 1536], BF16,
                                                  tag="probs", bufs=6,
                                                  name=f"pr{j}_{h}_{grp[0]}")
                                probs_of[h] = probs
                                nc.scalar.activation(
                                    probs[:, 0:ncols], sct[h][:, 0:ncols],
                                    AF.Exp, bias=0.0, scale=0.125)
                                # zero above-diagonal triangles (diag chunks)
                                for i, c in enumerate(grp):
                                    t = c - 4 * j
                                    if t < 0:
                                        continue
                                    if t == 3:
                                        nc.gpsimd.memset(
                                            probs[:, 512 * i + 256:
                                                  512 * i + 384], 0.0)
                                    nc.gpsimd.affine_select(
                                        out=probs[:, 512 * i + 128 * t:
                                                  512 * i + 128 * t + 128],
                                        in_=probs[:, 512 * i + 128 * t:
                                                  512 * i + 128 * t + 128],
                                        compare_op=ALU.is_ge, fill=0.0,
                                        base=0, pattern=[[1, 128]],
                                        channel_multiplier=-1)
                            if prev is not None:
                                emit_pv(*prev)
                            prev = (grp, probs_of)
                        emit_pv(*prev)
                        # normalize: attn = pv[0:64] / rowsum (pv row 64)
                        for h in heads:
                            rsum = pp.tile([1, 512], F32, tag="rsum",
                                             bufs=2, name=f"rs{j}_{h}")
                            nc.vector.tensor_copy(rsum[:], pv[h][64:65, :])
                            rcp = pp.tile([1, 512], F32, tag="rcp", bufs=2,
                                            name=f"rcp{j}_{h}")
                            nc.vector.reciprocal_approx_fast(
                                out=rcp[:], in_=rsum[:])
                            rb = pp.tile([64, 512], F32, tag="rb", bufs=2,
                                           name=f"rb{j}_{h}")
                            nc.gpsimd.partition_broadcast(rb[:], rcp[:])
                            dst = attn_all[h // 2][
                                64 * (h % 2):64 * (h % 2) + 64, qs]
                            nc.vector.tensor_tensor(dst, pv[h][0:64, :],
                                                    rb[:], ALU.mult)
                        # interleave previous block's o_proj into PE bubbles
                        if j >= 1:
                            emit_oproj(j - 1,
                                       [[0, 1], [2, 3], [4, 5], [6]][ip])
                if DEBUG:
                    dstage3 = pp.tile([128, S], F32, tag="dstage3",
                                        name="dstage3")
                    for i in range(4):
                        nc.vector.tensor_copy(dstage3[:], attn_all[i][:])
                        nc.sync.dma_start(
                            dbg["dattn"][128 * i:128 * i + 128, :],
                            dstage3[:])
                emit_oproj(3, list(range(7)))

    nc.compile()
    return nc


def _host_prep(inputs):
    import ml_dtypes
    BF = ml_dtypes.bfloat16
    hid = np.ascontiguousarray(np.asarray(inputs["hidden_states"], np.float32))
    pos = np.asarray(inputs["position_ids"])[0].astype(np.float32)
    Wq = np.asarray(inputs["Wq"], np.float32)
    bq = np.asarray(inputs["bq"], np.float32)
    Wk = np.asarray(inputs["Wk"], np.float32)
    bk = np.asarray(inputs["bk"], np.float32)
    Wv = np.asarray(inputs["Wv"], np.float32)
    bv = np.asarray(inputs["bv"], np.float32)
    Wo = np.asarray(inputs["Wo"], np.float32)

    inv = (1.0 / (ROPE_THETA ** (np.arange(0, HD, 2, dtype=np.float32) / HD))
           ).astype(np.float32)
    freqs = pos[:, None] * inv[None, :]
    emb = np.concatenate([freqs, freqs], -1)            # [S, 64]
    cosT = np.cos(emb).T.astype(np.float32)             # [64, S]
    sinT = np.sin(emb).T.astype(np.float32)
    sinm = sinT.copy()
    sinm[0:32] *= -1.0                                  # fold rotate_half sign
    cos2 = np.ascontiguousarray(np.vstack([cosT, cosT])).astype(BF)
    sinm2 = np.ascontiguousarray(np.vstack([sinm, sinm])).astype(BF)

    maps = []
    for b in range(B):
        for g in range(2):
            xT = np.ascontiguousarray(hid[b].T).astype(BF)
            # column blocks: [k(64) | q6(64) | q0..q5(384) | v(64)]
            Wsl = np.concatenate([Wk[64 * g:64 * g + 64],
                                  Wq[448 * g + 384:448 * g + 448],
                                  Wq[448 * g:448 * g + 384],
                                  Wv[64 * g:64 * g + 64]], 0)
            wT = np.ascontiguousarray(Wsl.T).astype(BF)  # [896, 576]
            bias = np.zeros(640, np.float32)
            bias[:576] = np.concatenate([bk[64 * g:64 * g + 64],
                                         bq[448 * g + 384:448 * g + 448],
                                         bq[448 * g:448 * g + 384],
                                         bv[64 * g:64 * g + 64]])
            woT = np.ascontiguousarray(
                Wo[:, 448 * g:448 * g + 448].T).astype(BF)
            maps.append(dict(xT=xT, wT=wT, bias=bias, woT=woT,
                             cos2=cos2, sinm2=sinm2,
                             ident64=np.eye(64, dtype=BF)))
    return maps


def kernel(**inputs) -> np.ndarray:
    from concourse.bass_utils import run_bass_kernel_spmd

    if "nc" not in _PROGRAM_CACHE:
        _PROGRAM_CACHE["nc"] = _build_program()
    nc = _PROGRAM_CACHE["nc"]

    in_maps = _host_prep(inputs)
    res = run_bass_kernel_spmd(nc, in_maps, core_ids=list(range(8)),
                               **_PROGRAM_CACHE.get("run_kwargs", {}))
    _PROGRAM_CACHE["last_result"] = res
    yTs = [res.results[i]["yT"] for i in range(8)]
    out = np.stack([(yTs[2 * b] + yTs[2 * b + 1]).T for b in range(B)], 0)
    return np.ascontiguousarray(out)
